# revision 2
# baseline (speedup 1.0000x reference)
"""MoE FFN (top-2 routing, 8 experts) on 8 Trainium2 NeuronCores.

Strategy (expert parallelism, per the sharding hint):
  - Host computes router logits / top-2 / softmax (tiny: T x E) and
    dispatches tokens: expert e's tokens are gathered into a padded
    [H, C] batch for core e (C = common capacity, multiple of 128).
  - Core e runs the dense FFN for its expert on its gathered tokens:
        yT = wt ⊙ ( GELU_tanh(x @ W1 + b1) @ W2 + b2 )^T
    computed fully transposed ([F,C] then [H,C]) so both matmuls use
    the weights as the stationary operand and no on-device transposes
    are needed. The per-token combine weight wt is folded in on-device.
  - Host scatter-adds each core's [H, C] result back into [T, H].

Self-contained: hardcodes the problem shapes (H=768, F=3072, E=8, K=2).
"""

import numpy as np

H = 768
F = 3072
E = 8
K = 2
N_CORES = 8
P = 128
CHUNK = 512  # token-chunk width (matmul moving-operand free dim, fp32 max)


# ---------------------------------------------------------------------------
# Bass/Tile device kernel
# ---------------------------------------------------------------------------

def _build_bass(C, Hd=H, Fd=F):
    """Build + compile the per-core Bass program for capacity C."""
    from contextlib import ExitStack

    import concourse.bass as bass
    import concourse.tile as tile
    from concourse import bacc, mybir
    from concourse._compat import with_exitstack

    assert C % P == 0 and Hd % P == 0 and Fd % P == 0
    FM = Fd // P          # number of 128-row tiles of the F dim
    HK = Hd // P          # contraction tiles for x@W1
    HN = Hd // P          # output row tiles of yT
    dt = mybir.dt.float32

    chunks = []
    c0 = 0
    while c0 < C:
        w = min(CHUNK, C - c0)
        chunks.append((c0, w))
        c0 += w

    nc = bacc.Bacc("TRN2", target_bir_lowering=False, debug=False,
                   num_devices=N_CORES)
    xgT = nc.dram_tensor("xgt", [Hd, C], dt, kind="ExternalInput").ap()
    w1 = nc.dram_tensor("w1", [Hd, Fd], dt, kind="ExternalInput").ap()
    b1t = nc.dram_tensor("b1t", [P, FM], dt, kind="ExternalInput").ap()
    w2 = nc.dram_tensor("w2", [Fd, Hd], dt, kind="ExternalInput").ap()
    b2r = nc.dram_tensor("b2r", [1, Hd], dt, kind="ExternalInput").ap()
    wtb = nc.dram_tensor("wtb", [P, C], dt, kind="ExternalInput").ap()
    y = nc.dram_tensor("y", [Hd, C], dt, kind="ExternalOutput").ap()

    gelu = mybir.ActivationFunctionType.Gelu_apprx_tanh

    @with_exitstack
    def body(ctx: ExitStack, tc: tile.TileContext):
        const = ctx.enter_context(tc.tile_pool(name="const", bufs=1))
        w1p = ctx.enter_context(tc.tile_pool(name="w1p", bufs=1))
        xp = ctx.enter_context(tc.tile_pool(name="xp", bufs=2))
        hrawp = ctx.enter_context(tc.tile_pool(name="hraw", bufs=2))
        hp = ctx.enter_context(tc.tile_pool(name="hp", bufs=1))
        w2p = ctx.enter_context(tc.tile_pool(name="w2p", bufs=4))
        yp = ctx.enter_context(tc.tile_pool(name="yp", bufs=3))
        psAp = ctx.enter_context(tc.tile_pool(name="psA", bufs=2, space="PSUM"))
        psBp = ctx.enter_context(tc.tile_pool(name="psB", bufs=1, space="PSUM"))

        # Constants / weights resident in SBUF
        b1s = const.tile([P, FM], dt)
        nc.sync.dma_start(b1s[:], b1t[:])
        b2s = const.tile([1, Hd], dt)
        nc.sync.dma_start(b2s[:], b2r[:])
        wtbs = const.tile([P, C], dt)
        nc.sync.dma_start(wtbs[:], wtb[:])
        w1t = []
        for k in range(HK):
            t = w1p.tile([P, Fd], dt, tag=f"w1_{k}")
            nc.sync.dma_start(t[:], w1[k * P:(k + 1) * P, :])
            w1t.append(t)

        for (c0, w) in chunks:
            # ---- load gathered-token chunk (transposed: [H, w]) ----
            xg = []
            for k in range(HK):
                t = xp.tile([P, CHUNK], dt, tag=f"xg{k}")
                nc.sync.dma_start(t[:, :w], xgT[k * P:(k + 1) * P, c0:c0 + w])
                xg.append(t)

            # ---- phase A: hT[f, c] = wt[c] * gelu((x@W1)[c, f] + b1[f]) ----
            hts = []
            for fm in range(FM):
                ps = psAp.tile([P, CHUNK], mybir.dt.float32, tag="psA")
                for hk in range(HK):
                    nc.tensor.matmul(
                        ps[:, :w],
                        lhsT=w1t[hk][:, fm * P:(fm + 1) * P],
                        rhs=xg[hk][:, :w],
                        start=(hk == 0), stop=(hk == HK - 1),
                    )
                hr = hrawp.tile([P, CHUNK], dt, tag="hraw")
                nc.scalar.activation(hr[:, :w], ps[:, :w], gelu,
                                     bias=b1s[:, fm:fm + 1])
                ht = hp.tile([P, CHUNK], dt, tag=f"hts{fm}")
                nc.vector.tensor_mul(ht[:, :w], hr[:, :w], wtbs[:, c0:c0 + w])
                hts.append(ht)

            # ---- phase B: yT[h, c] = sum_f W2[f, h] * hT[f, c] + wt[c]*b2[h]
            psBs = [psBp.tile([P, CHUNK], mybir.dt.float32, tag=f"psB{hn}",
                              name=f"psB{hn}")
                    for hn in range(HN)]
            for fk in range(FM):
                w2t = w2p.tile([P, Hd], dt, tag="w2t")
                nc.sync.dma_start(w2t[:], w2[fk * P:(fk + 1) * P, :])
                for hn in range(HN):
                    nc.tensor.matmul(
                        psBs[hn][:, :w],
                        lhsT=w2t[:, hn * P:(hn + 1) * P],
                        rhs=hts[fk][:, :w],
                        start=(fk == 0), stop=False,
                    )
            for hn in range(HN):
                # + b2 ⊗ wt  (rank-1; wt row gives the per-token scaling of b2)
                nc.tensor.matmul(
                    psBs[hn][:, :w],
                    lhsT=b2s[0:1, hn * P:(hn + 1) * P],
                    rhs=wtbs[0:1, c0:c0 + w],
                    start=False, stop=True,
                )
                ot = yp.tile([P, CHUNK], dt, tag="yout")
                nc.vector.tensor_copy(ot[:, :w], psBs[hn][:, :w])
                nc.sync.dma_start(y[hn * P:(hn + 1) * P, c0:c0 + w],
                                  ot[:, :w])

    with tile.TileContext(nc) as tc:
        body(tc)
    nc.compile()
    return nc


# ---------------------------------------------------------------------------
# Host-side routing + dispatch
# ---------------------------------------------------------------------------

def _route(xf, gate_w):
    """Top-2 router in float64 for a numerically robust top-k set.

    Returns per-expert (token_idx, weight) lists.
    """
    logits = xf.astype(np.float64) @ gate_w.astype(np.float64)  # [T, E]
    top_idx = np.argpartition(logits, E - K, axis=1)[:, E - K:]  # [T, K]
    top_val = np.take_along_axis(logits, top_idx, axis=1)
    m = top_val.max(axis=1, keepdims=True)
    ex = np.exp(top_val - m)
    wts = ex / ex.sum(axis=1, keepdims=True)  # [T, K] float64

    toks, ws = [], []
    for e in range(E):
        mask = top_idx == e  # [T, K]
        rows = np.nonzero(mask.any(axis=1))[0]
        toks.append(rows)
        ws.append(wts[mask].astype(np.float32))
    return toks, ws


def kernel(x, gate_w, W1, b1, W2, b2):
    from concourse.bass_utils import run_bass_kernel_spmd

    x = np.asarray(x)
    out_dtype = x.dtype
    Bb, S, Hd = x.shape
    assert Hd == H
    T = Bb * S
    xf = np.ascontiguousarray(x.reshape(T, Hd), dtype=np.float32)
    gate_w = np.asarray(gate_w, np.float32)
    W1a = np.asarray(W1, np.float32)
    b1a = np.asarray(b1, np.float32)
    W2a = np.asarray(W2, np.float32)
    b2a = np.asarray(b2, np.float32)

    toks, ws = _route(xf, gate_w)
    nmax = max(len(t) for t in toks)
    C = max(P, ((nmax + P - 1) // P) * P)

    nc = _build_bass(C)

    in_maps = []
    for e in range(E):
        n_e = len(toks[e])
        xgT = np.zeros((H, C), np.float32)
        xgT[:, :n_e] = xf[toks[e]].T
        wtb = np.zeros((P, C), np.float32)
        wtb[:, :n_e] = ws[e][None, :]
        in_maps.append({
            "xgt": xgT,
            "w1": np.ascontiguousarray(W1a[e]),
            "b1t": np.ascontiguousarray(b1a[e].reshape(F // P, P).T),
            "w2": np.ascontiguousarray(W2a[e]),
            "b2r": np.ascontiguousarray(b2a[e][None, :]),
            "wtb": wtb,
        })

    res = run_bass_kernel_spmd(nc, in_maps, core_ids=list(range(N_CORES)))
    out = np.zeros((T, H), np.float32)
    for e in range(E):
        n_e = len(toks[e])
        out[toks[e]] += res.results[e]["y"][:, :n_e].T
    return out.reshape(Bb, S, Hd).astype(out_dtype, copy=False)


# Exposed for test.py: run with profiling and return (output, BassKernelResults)
def kernel_profiled(x, gate_w, W1, b1, W2, b2):
    from concourse.bass_utils import run_bass_kernel_spmd

    x = np.asarray(x)
    Bb, S, Hd = x.shape
    T = Bb * S
    xf = np.ascontiguousarray(x.reshape(T, Hd), dtype=np.float32)
    toks, ws = _route(xf, np.asarray(gate_w, np.float32))
    nmax = max(len(t) for t in toks)
    C = max(P, ((nmax + P - 1) // P) * P)
    nc = _build_bass(C)

    W1a = np.asarray(W1, np.float32)
    b1a = np.asarray(b1, np.float32)
    W2a = np.asarray(W2, np.float32)
    b2a = np.asarray(b2, np.float32)
    in_maps = []
    for e in range(E):
        n_e = len(toks[e])
        xgT = np.zeros((H, C), np.float32)
        xgT[:, :n_e] = xf[toks[e]].T
        wtb = np.zeros((P, C), np.float32)
        wtb[:, :n_e] = ws[e][None, :]
        in_maps.append({
            "xgt": xgT,
            "w1": np.ascontiguousarray(W1a[e]),
            "b1t": np.ascontiguousarray(b1a[e].reshape(F // P, P).T),
            "w2": np.ascontiguousarray(W2a[e]),
            "b2r": np.ascontiguousarray(b2a[e][None, :]),
            "wtb": wtb,
        })
    res = run_bass_kernel_spmd(nc, in_maps, core_ids=list(range(N_CORES)),
                               trace=True, trace_cores=list(range(N_CORES)))
    out = np.zeros((T, H), np.float32)
    for e in range(E):
        n_e = len(toks[e])
        out[toks[e]] += res.results[e]["y"][:, :n_e].T
    return out.reshape(Bb, S, Hd), res


# revision 3
# speedup vs baseline: 3.2502x; 3.2502x over previous
"""MoE FFN (top-2 routing, 8 experts) on 8 Trainium2 NeuronCores.

Strategy (expert parallelism, per the sharding hint):
  - Host computes router logits / top-2 / softmax (tiny: T x E) and
    dispatches tokens: expert e's tokens are gathered into a padded
    [H, C] batch for core e (C = common capacity, multiple of 128).
  - Core e runs the dense FFN for its expert on its gathered tokens:
        yT = wt ⊙ ( GELU_tanh(x @ W1 + b1) @ W2 + b2 )^T
    computed fully transposed ([F,C] then [H,C]) so both matmuls use
    the weights as the stationary operand and no on-device transposes
    are needed. Matmul operands are bf16 (PE runs fp32 at 1/4 rate;
    accumulation stays fp32 in PSUM); biases, GELU, and the per-token
    combine weight are applied in fp32.
  - Host scatter-adds each core's [H, C] result back into [T, H].

Self-contained: hardcodes the problem shapes (H=768, F=3072, E=8, K=2).
"""

import os

import numpy as np

H = 768
F = 3072
E = 8
K = 2
N_CORES = 8
P = 128
CHUNK = 512  # token-chunk width (fp32 PSUM bank = 512 elems)

PRECISION = os.environ.get("MOE_PRECISION", "bf16")  # "bf16" | "fp32"


# ---------------------------------------------------------------------------
# Bass/Tile device kernel
# ---------------------------------------------------------------------------

def _build_bass(C, Hd=H, Fd=F, precision=None):
    """Build + compile the per-core Bass program for capacity C."""
    from contextlib import ExitStack

    import concourse.bass as bass  # noqa: F401
    import concourse.tile as tile
    from concourse import bacc, mybir
    from concourse._compat import with_exitstack

    precision = precision or PRECISION
    assert C % P == 0 and Hd % P == 0 and Fd % P == 0
    FM = Fd // P          # number of 128-row tiles of the F dim
    HK = Hd // P          # contraction tiles for x@W1
    HN = Hd // P          # output row tiles of yT
    f32 = mybir.dt.float32
    mdt = mybir.dt.bfloat16 if precision == "bf16" else f32

    chunks = []
    c0 = 0
    while c0 < C:
        w = min(CHUNK, C - c0)
        chunks.append((c0, w))
        c0 += w

    nc = bacc.Bacc("TRN2", target_bir_lowering=False, debug=False,
                   num_devices=N_CORES)
    xgT = nc.dram_tensor("xgt", [Hd, C], mdt, kind="ExternalInput").ap()
    w1 = nc.dram_tensor("w1", [Hd, Fd], mdt, kind="ExternalInput").ap()
    b1t = nc.dram_tensor("b1t", [P, FM], f32, kind="ExternalInput").ap()
    w2 = nc.dram_tensor("w2", [Fd, Hd], mdt, kind="ExternalInput").ap()
    b2c = nc.dram_tensor("b2c", [P, HN], f32, kind="ExternalInput").ap()
    wtb = nc.dram_tensor("wtb", [P, C], f32, kind="ExternalInput").ap()
    y = nc.dram_tensor("y", [Hd, C], f32, kind="ExternalOutput").ap()

    gelu = mybir.ActivationFunctionType.Gelu_apprx_tanh
    ident = mybir.ActivationFunctionType.Identity

    @with_exitstack
    def body(ctx: ExitStack, tc: tile.TileContext):
        const = ctx.enter_context(tc.tile_pool(name="const", bufs=1))
        w1p = ctx.enter_context(tc.tile_pool(name="w1p", bufs=1))
        xp = ctx.enter_context(tc.tile_pool(name="xp", bufs=2))
        hp = ctx.enter_context(tc.tile_pool(name="hp", bufs=1))
        w2p = ctx.enter_context(tc.tile_pool(name="w2p", bufs=4))
        yp = ctx.enter_context(tc.tile_pool(name="yp", bufs=3))
        psAp = ctx.enter_context(tc.tile_pool(name="psA", bufs=2, space="PSUM"))
        psBp = ctx.enter_context(tc.tile_pool(name="psB", bufs=1, space="PSUM"))

        # First chunk's tokens first so PE can start ASAP, then weights.
        xg0 = []
        c00, w0 = chunks[0]
        for k in range(HK):
            t = xp.tile([P, CHUNK], mdt, tag=f"xg{k}", name=f"xg0_{k}")
            nc.sync.dma_start(t[:, :w0], xgT[k * P:(k + 1) * P, c00:c00 + w0])
            xg0.append(t)
        w1t = []
        for k in range(HK):
            t = w1p.tile([P, Fd], mdt, tag=f"w1_{k}", name=f"w1_{k}")
            nc.sync.dma_start(t[:], w1[k * P:(k + 1) * P, :])
            w1t.append(t)
        b1s = const.tile([P, FM], f32)
        nc.sync.dma_start(b1s[:], b1t[:])
        b2s = const.tile([P, HN], f32)
        nc.sync.dma_start(b2s[:], b2c[:])
        wtbs = const.tile([P, C], f32)
        nc.sync.dma_start(wtbs[:], wtb[:])

        for ci, (c0, w) in enumerate(chunks):
            # ---- load gathered-token chunk (transposed: [H, w]) ----
            if ci == 0:
                xg = xg0
            else:
                xg = []
                for k in range(HK):
                    t = xp.tile([P, CHUNK], mdt, tag=f"xg{k}",
                                name=f"xg{ci}_{k}")
                    nc.sync.dma_start(t[:, :w],
                                      xgT[k * P:(k + 1) * P, c0:c0 + w])
                    xg.append(t)

            # ---- phase A: hT[f, c] = gelu((x@W1)[c, f] + b1[f]) ----
            hts = []
            for fm in range(FM):
                ps = psAp.tile([P, CHUNK], f32, tag="psA", name="psA")
                for hk in range(HK):
                    nc.tensor.matmul(
                        ps[:, :w],
                        lhsT=w1t[hk][:, fm * P:(fm + 1) * P],
                        rhs=xg[hk][:, :w],
                        start=(hk == 0), stop=(hk == HK - 1),
                    )
                ht = hp.tile([P, CHUNK], mdt, tag=f"hts{fm}", name=f"hts{fm}")
                nc.scalar.activation(ht[:, :w], ps[:, :w], gelu,
                                     bias=b1s[:, fm:fm + 1])
                hts.append(ht)

            # ---- phase B: yT[h, c] = sum_f W2[f, h] * hT[f, c] ----
            psBs = [psBp.tile([P, CHUNK], f32, tag=f"psB{hn}",
                              name=f"psB{hn}")
                    for hn in range(HN)]
            for fk in range(FM):
                w2t = w2p.tile([P, Hd], mdt, tag="w2t", name="w2t")
                nc.sync.dma_start(w2t[:], w2[fk * P:(fk + 1) * P, :])
                for hn in range(HN):
                    nc.tensor.matmul(
                        psBs[hn][:, :w],
                        lhsT=w2t[:, hn * P:(hn + 1) * P],
                        rhs=hts[fk][:, :w],
                        start=(fk == 0), stop=(fk == FM - 1),
                    )
            # ---- epilogue: (+b2), (*wt), store ----
            for hn in range(HN):
                ot = yp.tile([P, CHUNK], f32, tag="yout", name="yout")
                nc.scalar.activation(ot[:, :w], psBs[hn][:, :w], ident,
                                     bias=b2s[:, hn:hn + 1])
                ot2 = yp.tile([P, CHUNK], f32, tag="yout2", name="yout2")
                nc.vector.tensor_mul(ot2[:, :w], ot[:, :w],
                                     wtbs[:, c0:c0 + w])
                nc.sync.dma_start(y[hn * P:(hn + 1) * P, c0:c0 + w],
                                  ot2[:, :w])

    with tile.TileContext(nc) as tc:
        body(tc)
    nc.compile()
    return nc


# ---------------------------------------------------------------------------
# Host-side routing + dispatch
# ---------------------------------------------------------------------------

def _route(xf, gate_w):
    """Top-2 router in float64 for a numerically robust top-k set.

    Returns per-expert (token_idx, weight) lists.
    """
    logits = xf.astype(np.float64) @ gate_w.astype(np.float64)  # [T, E]
    top_idx = np.argpartition(logits, E - K, axis=1)[:, E - K:]  # [T, K]
    top_val = np.take_along_axis(logits, top_idx, axis=1)
    m = top_val.max(axis=1, keepdims=True)
    ex = np.exp(top_val - m)
    wts = ex / ex.sum(axis=1, keepdims=True)  # [T, K] float64

    toks, ws = [], []
    for e in range(E):
        mask = top_idx == e  # [T, K]
        rows = np.nonzero(mask.any(axis=1))[0]
        toks.append(rows)
        ws.append(wts[mask].astype(np.float32))
    return toks, ws


def _np_mdt():
    import ml_dtypes
    return ml_dtypes.bfloat16 if PRECISION == "bf16" else np.float32


def _make_in_maps(xf, gate_w, W1, b1, W2, b2):
    toks, ws = _route(xf, gate_w)
    nmax = max(len(t) for t in toks)
    C = max(P, ((nmax + P - 1) // P) * P)
    mdt = _np_mdt()

    W1a = np.asarray(W1, np.float32)
    b1a = np.asarray(b1, np.float32)
    W2a = np.asarray(W2, np.float32)
    b2a = np.asarray(b2, np.float32)
    in_maps = []
    for e in range(E):
        n_e = len(toks[e])
        xgT = np.zeros((H, C), mdt)
        xgT[:, :n_e] = xf[toks[e]].T.astype(mdt)
        wtb = np.zeros((P, C), np.float32)
        wtb[:, :n_e] = ws[e][None, :]
        in_maps.append({
            "xgt": xgT,
            "w1": W1a[e].astype(mdt),
            "b1t": np.ascontiguousarray(b1a[e].reshape(F // P, P).T),
            "w2": W2a[e].astype(mdt),
            "b2c": np.ascontiguousarray(b2a[e].reshape(H // P, P).T),
            "wtb": wtb,
        })
    return in_maps, toks, C


def _run(inputs, trace=False):
    from concourse.bass_utils import run_bass_kernel_spmd

    x, gate_w, W1, b1, W2, b2 = (inputs[k] for k in
                                 ("x", "gate_w", "W1", "b1", "W2", "b2"))
    x = np.asarray(x)
    Bb, S, Hd = x.shape
    assert Hd == H
    T = Bb * S
    xf = np.ascontiguousarray(x.reshape(T, Hd), dtype=np.float32)
    gate_w = np.asarray(gate_w, np.float32)

    in_maps, toks, C = _make_in_maps(xf, gate_w, W1, b1, W2, b2)
    nc = _build_bass(C)

    kwargs = {}
    if trace:
        kwargs = dict(trace=True, trace_cores=list(range(N_CORES)))
    res = run_bass_kernel_spmd(nc, in_maps, core_ids=list(range(N_CORES)),
                               **kwargs)
    out = np.zeros((T, H), np.float32)
    for e in range(E):
        n_e = len(toks[e])
        out[toks[e]] += res.results[e]["y"][:, :n_e].T
    return out.reshape(Bb, S, Hd), res


def kernel(x, gate_w, W1, b1, W2, b2):
    out, _ = _run({"x": x, "gate_w": gate_w, "W1": W1, "b1": b1,
                   "W2": W2, "b2": b2})
    return out.astype(np.asarray(x).dtype, copy=False)


# Exposed for test.py: run with profiling, return (output, BassKernelResults)
def kernel_profiled(x, gate_w, W1, b1, W2, b2):
    return _run({"x": x, "gate_w": gate_w, "W1": W1, "b1": b1,
                 "W2": W2, "b2": b2}, trace=True)
